# revision 31
# baseline (speedup 1.0000x reference)
"""Distributional twin-critic MLP forward, data-parallel over 8 NeuronCores.

Math (per critic c, eval mode):
    x   = concat(state, action)                       [B, 576]
    h   = relu(LN(x @ W_f1.T + b_f1) * g1 + beta1)    [B, 1024]
    f   = relu(LN(h @ W_f2.T + b_f2) * g2 + beta2)    [B, 1024]
    q   = f @ wh_feat + te @ wh_tau + b_h             [B, NQ] (outer sum)

Device strategy (pure data parallel, batch shard 2048 rows per core), fast
path (g==1, beta==0, which is what setup_inputs produces):
  - all main matmuls in bf16 with power-of-2 operand scales (x8 on x/w1/
    w2/wh); bf16 streams at the same 1 cyc/row as fp32r but gets Fast
    Weight Load (contiguous 2-byte weights), halving the LDWEIGHTS
    exposure that dominated the fp32r version's non-stream PE time, and
    halves the HBM prologue.  fp8 (measured in simulation) busts the 2e-2
    gate even for a single quantized operand (~2.5e-2 max rel), so
    DoubleRow is used ONLY for the LN-stats matmuls, whose 1/H averaging
    kills the quantization noise: z^2 is written as e4m3 [128,2,NT] pair
    tiles and contracted 256 features per pass.
  - LayerNorm mean is folded into the weights host-side (centering
    preserves the math exactly), so on-chip LN is an RMS-norm; E[z^2]
    accumulates via all-ones fp8 DoubleRow stationary matmuls whose M=128
    output also broadcasts the stats across partitions; rstd comes from
    the scalar-engine 1/sqrt(|x|) table with the power-of-2 scale folded
    into the activation's scale/bias immediates.
  - biases never ride a ones-row: layer biases are applied as per-
    partition operands of the ACT square (bias) and DVE relu
    (tensor_scalar add+max) that read the matmul PSUM anyway.  This
    frees the layer-1 K tail to be exactly the 64 action rows, and the
    two critics' K=64 action matmuls run CONCURRENTLY in disjoint halves
    of the PE array via tile_position row tiling.
  - rstd1 is NOT multiplied into h (per-sample scale commutes through
    the layer-2 matmul): h = relu(z1+b1) in bf16 straight from PSUM, and
    rstd1 lands in the single per-m-tile DVE multiply that converts
    layer-2 PSUM to bf16 (tmp), halving the DVE/ACT passes vs
    normalizing h in place.  rstd2 similarly lands on the [64,NT] head
    output (RMS-norm scale invariance).
  - tau embedding is batch-independent -> computed on host (64x64 chain).
  - head: wh replicated to 64 psum partitions; output written nq-major
    [2, 64, B_shard] and transposed on the host during the gather.
  - a general g/beta fallback variant (the previous fp32r kernel) is
    built if the affine params are not identity.
"""

import os
import sys

import numpy as np

sys.path.insert(0, "/opt/trn_rl_repo")

import concourse.bacc as bacc
import concourse.tile as tile
from concourse import mybir
from concourse.bass_utils import run_bass_kernel_spmd

F32 = mybir.dt.float32
F32R = mybir.dt.float32r
BF16 = mybir.dt.bfloat16
E4 = mybir.dt.float8e4
AF = mybir.ActivationFunctionType
ALU = mybir.AluOpType
DR = mybir.MatmulPerfMode.DoubleRow

B, SD, AD, H, QE, NQ = 16384, 512, 64, 1024, 64, 64
D = SD + AD                      # 576
DA = D + 1                       # ones-row variant (general path only)
NCORES = 8
BSH = B // NCORES                # 2048 batch rows per core
NT = 512                         # batch tile (matmul free dim)
NBT = BSH // NT                  # 4
NM = H // 128                    # 8 M-tiles (and K-tiles for layer 2)
EPS = 1e-5
KL1 = [128, 128, 128, 128, 65]   # general path K tiling

_CACHE = {}
_LAST_RESULT = None


def _build_fast(nc):
    # x pre-tiled per batch tile so every DMA line is per-partition
    # contiguous (strided x lines measured ~30 GB/s vs ~300 contiguous)
    xs = nc.dram_tensor("xs", [NBT, 128, 4, NT], BF16,
                        kind="ExternalInput").ap()
    xa = nc.dram_tensor("xa", [NBT, 128, NT], BF16,
                        kind="ExternalInput").ap()
    w1s = nc.dram_tensor("w1s", [2, 128, 4, H], BF16,
                         kind="ExternalInput").ap()
    w1a = nc.dram_tensor("w1a", [2, 128, H], BF16, kind="ExternalInput").ap()
    w2 = nc.dram_tensor("w2", [2, 2, 128, 4, H], BF16,
                        kind="ExternalInput").ap()
    wh = nc.dram_tensor("wh", [2, 128, NM, 64], BF16,
                        kind="ExternalInput").ap()
    # per-feature vectors [c, p, vec, m], feature = m*128+p;
    # vec order: 4*b1c, 64*b1c, 4*b2c, 64*b2c
    vecs = nc.dram_tensor("vecs", [2, 128, 4, NM], F32,
                          kind="ExternalInput").ap()
    qtb = nc.dram_tensor("qtb", [2, 64, 1], F32, kind="ExternalInput").ap()
    out_q = nc.dram_tensor("out_q", [2, NQ, BSH], F32,
                           kind="ExternalOutput").ap()

    with tile.TileContext(nc) as tc:
        with tc.tile_pool(name="wpool", bufs=1) as wp, \
             tc.tile_pool(name="xpool", bufs=2) as xp, \
             tc.tile_pool(name="sqpool", bufs=2) as qp_, \
             tc.tile_pool(name="hpool", bufs=2) as hp, \
             tc.tile_pool(name="tpool", bufs=1) as tp_, \
             tc.tile_pool(name="spool", bufs=2) as sp_, \
             tc.tile_pool(name="zpsum", bufs=2, space="PSUM") as zps, \
             tc.tile_pool(name="spsum", bufs=1, space="PSUM") as sps, \
             tc.tile_pool(name="qpsum", bufs=2, space="PSUM") as qps:

            # ---- resident weights, consumption order ----
            vt = [wp.tile([128, 4, NM], F32, tag=f"vec_{c}", name=f"vec_{c}")
                  for c in range(2)]
            qtbv = [wp.tile([64, 1], F32, tag=f"qtb_{c}", name=f"qtb_{c}")
                    for c in range(2)]

            # prologue DMA across the 3 hw queues, gating chunks (x on sync
            # first, then w1 both critics split) so no queue serializes >
            # ~1.3MB ahead of the first matmuls; w2/wh follow in
            # consumption order.
            def x_tiles(bt):
                xst = xp.tile([128, 4, NT], BF16, tag="xs", name="xs")
                nc.sync.dma_start(out=xst[:], in_=xs[bt])
                xat = xp.tile([128, NT], BF16, tag="xa", name="xa")
                nc.sync.dma_start(out=xat[:], in_=xa[bt])
                return xst, xat

            # few, large prologue transfers (each trigger costs ~0.7us of
            # engine time, serially per queue); gating pieces lead each queue
            x0 = x_tiles(0)
            w1t = [wp.tile([128, 4, H], BF16, tag=f"w1_{c}", name=f"w1_{c}")
                   for c in range(2)]
            w1at = [wp.tile([128, H], BF16, tag=f"w1a_{c}", name=f"w1a_{c}")
                    for c in range(2)]
            w2t = [[wp.tile([128, 4, H], BF16, tag=f"w2_{c}_{j}",
                            name=f"w2_{c}_{j}") for j in range(2)]
                   for c in range(2)]
            wht = [wp.tile([128, NM, 64], BF16, tag=f"wh_{c}", name=f"wh_{c}")
                   for c in range(2)]
            nc.gpsimd.dma_start(out=w1t[0][:, 0:2, :], in_=w1s[0, :, 0:2, :])
            nc.scalar.dma_start(out=w1t[0][:, 2:4, :], in_=w1s[0, :, 2:4, :])
            nc.scalar.dma_start(out=w1at[0][:], in_=w1a[0])
            nc.gpsimd.dma_start(out=w1at[1][:], in_=w1a[1])
            nc.sync.dma_start(out=w1t[1][:], in_=w1s[1])
            for c in range(2):
                nc.gpsimd.dma_start(out=vt[c][:], in_=vecs[c])
                nc.gpsimd.dma_start(out=qtbv[c][:], in_=qtb[c])
            nc.gpsimd.dma_start(out=w2t[0][0][:], in_=w2[0, 0])
            nc.scalar.dma_start(out=w2t[0][1][:], in_=w2[0, 1])
            nc.sync.dma_start(out=w2t[1][0][:], in_=w2[1, 0])
            nc.scalar.dma_start(out=w2t[1][1][:], in_=w2[1, 1])
            nc.gpsimd.dma_start(out=wht[0][:], in_=wh[0])
            nc.scalar.dma_start(out=wht[1][:], in_=wh[1])

            # stats stationary: all-ones fp8 DoubleRow pair tile
            mt8 = wp.tile([128, 2, 128], E4, tag="mt8", name="mt8")
            nc.vector.memset(mt8[:], 1.0)
            ep1 = wp.tile([128, 1], F32, tag="ep1", name="ep1")
            nc.vector.memset(ep1[:], 64.0 * EPS)
            ep2 = wp.tile([64, 1], F32, tag="ep2", name="ep2")
            nc.vector.memset(ep2[:], 262144.0 * EPS)
            # HAM warmup while the first weights stream in; must run gapless
            # into the first real matmuls or the 4us-continuous-busy window
            # resets and the first ~8us of real matmuls run throttled
            warm = wp.tile([128, 128], BF16, tag="warm", name="warm")
            nc.vector.memset(warm[:], 1.0)
            wmp = qps.tile([128, 128], F32, tag="qp", name="wmp")
            for _ in range(52):
                nc.tensor.matmul(wmp[:], warm[:], warm[:], start=True,
                                 stop=True)

            def w1_ap(c, k, m):
                return w1t[c][:, k, m * 128:(m + 1) * 128]

            def w2_ap(c, k, m):
                return w2t[c][k // 4][:, k % 4, m * 128:(m + 1) * 128]

            def b_ap(c, i, m):
                return vt[c][:, i, m:m + 1]

            # trailing stats matmuls + their rsqrt/head-epilogue chains are
            # deferred into the NEXT block's matmul stream: issued in place
            # they stall the in-order PE ~1.2us waiting on the ACT square
            # of the block's last m-tile.
            deferred = []

            def run_deferred():
                while deferred:
                    deferred.pop(0)()

            for bt in range(NBT):
                b0 = bt * NT
                xst, xat = x0 if bt == 0 else x_tiles(bt)

                # -------- layer 1, critic-serial; action matmuls of each
                # even/odd m pair share the PE array via row tiling --------
                hs = [[hp.tile([128, NT], BF16, tag=f"h_{c}_{m}",
                               name=f"h_{c}_{m}") for m in range(NM)]
                      for c in range(2)]
                rs1 = {}
                for c in range(2):
                    sq1 = [qp_.tile([128, 2, NT], E4, tag=f"sq1_{c}_{kp}",
                                    name=f"sq1_{c}_{kp}") for kp in range(4)]
                    sp1 = sps.tile([128, NT], F32, tag=f"sp_{c}",
                                   name=f"sp1_{c}")

                    def stats1(kp, c=c, sp1=sp1, sq1=sq1):
                        nc.tensor.matmul(sp1[:], mt8[:], sq1[kp][:],
                                         start=(kp == 0), stop=(kp == 3),
                                         perf_mode=DR)

                    def rsq1(c=c, sp1=sp1):
                        r = sp_.tile([128, NT], F32, tag=f"rs1_{c}",
                                     name=f"rs1_{c}")
                        # 1/sqrt(sum(16 z^2)/256 + 64 eps) = rstd1/8
                        nc.scalar.activation(r[:], sp1[:],
                                             AF.Abs_reciprocal_sqrt,
                                             bias=ep1[:], scale=1.0 / 256)
                        rs1[c] = r

                    for mp in range(0, NM, 2):
                        zpm = []
                        for m in (mp, mp + 1):
                            z = zps.tile([128, NT], F32, tag=f"z_{m % 2}",
                                         name=f"z1_{c}_{m}")
                            for k in range(4):
                                nc.tensor.matmul(z[:], w1_ap(c, k, m),
                                                 xst[:, k, :], start=(k == 0),
                                                 stop=False)
                            zpm.append(z)
                        for i, m in enumerate((mp, mp + 1)):
                            nc.tensor.matmul(
                                zpm[i][:], w1at[c][i * 64:(i + 1) * 64,
                                                   m * 128:(m + 1) * 128],
                                xat[i * 64:(i + 1) * 64, :], start=False,
                                stop=True, tile_position=(i * 64, 0))
                        if mp == 0:
                            run_deferred()
                        for i, m in enumerate((mp, mp + 1)):
                            # sq = (z/16 + 4*b1c)^2 = 16*z1c^2 [ACT psum->fp8]
                            nc.scalar.activation(sq1[m // 2][:, m % 2, :],
                                                 zpm[i][:], AF.Square,
                                                 bias=b_ap(c, 0, m),
                                                 scale=1.0 / 16)
                            # h = max(z + 64*b1c, 0) = 64*relu(z1c) [DVE]
                            nc.vector.tensor_scalar(hs[c][m][:], zpm[i][:],
                                                    b_ap(c, 1, m), 0.0,
                                                    ALU.add, ALU.max)
                        if mp >= 4:
                            stats1(mp // 2 - 2)
                    stats1(2)
                    deferred.append(lambda s=stats1, r=rsq1: (s(3), r()))

                # ---------------- layer 2 + head, per critic ------------
                for c in range(2):
                    sq2 = [qp_.tile([128, 2, NT], E4, tag=f"sq2_{c}_{kp}",
                                    name=f"sq2_{c}_{kp}") for kp in range(4)]
                    fs = [hp.tile([128, NT], BF16, tag=f"f_{c}_{m}",
                                  name=f"f_{c}_{m}") for m in range(NM)]
                    sp2 = sps.tile([128, NT], F32, tag=f"sp_{c}",
                                   name=f"sp2_{c}")

                    def stats2(kp, sp2=sp2, sq2=sq2):
                        nc.tensor.matmul(sp2[:], mt8[:], sq2[kp][:],
                                         start=(kp == 0), stop=(kp == 3),
                                         perf_mode=DR)

                    for m in range(NM):
                        z = zps.tile([128, NT], F32, tag=f"z_{m % 2}",
                                     name=f"z2_{c}_{m}")
                        for k in range(NM):
                            nc.tensor.matmul(z[:], w2_ap(c, k, m),
                                             hs[c][k][:], start=(k == 0),
                                             stop=(k == NM - 1))
                        if m == 1:
                            run_deferred()
                        # tmp = psum * rs1 = 64*(z2c - b2c)    [DVE, ->bf16]
                        tmp = tp_.tile([128, NT], BF16, tag=f"tmp{m % 3}",
                                       name=f"tmp_{c}_{m}", bufs=1)
                        nc.vector.tensor_tensor(tmp[:], z[:], rs1[c][:],
                                                op=ALU.mult)
                        # sq = (tmp/16 + 4*b2c)^2 = 16*z2c^2   [ACT, ->fp8]
                        nc.scalar.activation(sq2[m // 2][:, m % 2, :], tmp[:],
                                             AF.Square, bias=b_ap(c, 2, m),
                                             scale=1.0 / 16)
                        # f = max(tmp + 64*b2c, 0) = 64*relu(z2c)  [DVE]
                        nc.vector.tensor_scalar(fs[m][:], tmp[:],
                                                b_ap(c, 3, m), 0.0,
                                                ALU.add, ALU.max)
                        if m >= 5 and m % 2 == 1:
                            stats2((m - 5) // 2)
                    stats2(2)
                    qp = qps.tile([64, NT], F32, tag="qp", name=f"qp_{c}")
                    for k in range(NM):
                        nc.tensor.matmul(qp[:], wht[c][:, k, :], fs[k][:],
                                         start=(k == 0), stop=(k == NM - 1))

                    def epilogue(c=c, sp2=sp2, qp=qp, s2=stats2, b0=b0):
                        s2(3)
                        rs2 = sp_.tile([64, NT], F32, tag="rs2",
                                       name=f"rs2_{c}", bufs=2)
                        # 1/sqrt(16*sum(16 z^2) + 2.62144) = rstd2/512
                        nc.scalar.activation(rs2[:], sp2[0:64, :],
                                             AF.Abs_reciprocal_sqrt,
                                             bias=ep2[:], scale=16.0)
                        q0 = sp_.tile([64, NT], F32, tag="q0",
                                      name=f"q0_{c}", bufs=2)
                        nc.vector.tensor_tensor(q0[:], qp[:], rs2[:],
                                                op=ALU.mult)
                        qf = sp_.tile([64, NT], F32, tag="qf",
                                      name=f"qf_{c}", bufs=2)
                        nc.scalar.activation(qf[:], q0[:], AF.Identity,
                                             bias=qtbv[c][:])
                        nc.gpsimd.dma_start(out=out_q[c, :, b0:b0 + NT],
                                            in_=qf[:])

                    deferred.append(epilogue)
            run_deferred()
    nc.compile()
    return nc


def _build_general(nc):
    """Previous fp32r kernel (general g/beta path) — unchanged."""
    from concourse.masks import make_identity  # noqa: F401

    xT = nc.dram_tensor("xT", [DA, BSH], F32R, kind="ExternalInput").ap()
    w1 = nc.dram_tensor("w1", [2, DA, H], F32R, kind="ExternalInput").ap()
    w2 = nc.dram_tensor("w2", [2, H, H], F32R, kind="ExternalInput").ap()
    whr = nc.dram_tensor("whr", [2, H, 64], F32R, kind="ExternalInput").ap()
    vecs = nc.dram_tensor("vecs", [2, 128, 6, NM], F32,
                          kind="ExternalInput").ap()
    qtb = nc.dram_tensor("qtb", [2, 64, 1], F32, kind="ExternalInput").ap()
    out_q = nc.dram_tensor("out_q", [2, NQ, BSH], F32,
                           kind="ExternalOutput").ap()

    with tile.TileContext(nc) as tc:
        with tc.tile_pool(name="wpool", bufs=1) as wp, \
             tc.tile_pool(name="xpool", bufs=2) as xp, \
             tc.tile_pool(name="zpool", bufs=2) as zp_, \
             tc.tile_pool(name="hpool", bufs=2) as hp, \
             tc.tile_pool(name="spool", bufs=2) as sp_, \
             tc.tile_pool(name="zpsum", bufs=6, space="PSUM") as zps, \
             tc.tile_pool(name="spsum", bufs=1, space="PSUM") as sps, \
             tc.tile_pool(name="qpsum", bufs=1, space="PSUM") as qps:

            w1b = [wp.tile([128, 4, H], F32R, tag=f"w1b_{c}", name=f"w1b_{c}")
                   for c in range(2)]
            w1x = [wp.tile([KL1[4], H], F32R, tag=f"w1x_{c}",
                           name=f"w1x_{c}") for c in range(2)]
            w2b = [[wp.tile([128, 4, H], F32R, tag=f"w2b_{c}_{j}",
                            name=f"w2b_{c}_{j}") for j in range(2)]
                   for c in range(2)]
            wht = [wp.tile([128, NM, 64], F32R, tag=f"wh_{c}", name=f"wh_{c}")
                   for c in range(2)]
            vt = [wp.tile([128, 6, NM], F32, tag=f"vec_{c}", name=f"vec_{c}")
                  for c in range(2)]
            qtbv = [wp.tile([64, 1], F32, tag=f"qtb_{c}", name=f"qtb_{c}")
                    for c in range(2)]

            def w1_ap(c, k, m):
                if k < 4:
                    return w1b[c][:, k, m * 128:(m + 1) * 128]
                return w1x[c][:, m * 128:(m + 1) * 128]

            def w2_ap(c, k, m):
                return w2b[c][k // 4][:, k % 4, m * 128:(m + 1) * 128]

            for k in range(4):
                eng = nc.gpsimd if k % 2 == 0 else nc.scalar
                eng.dma_start(out=w1b[0][:, k, :],
                              in_=w1[0, k * 128:(k + 1) * 128, :])
            nc.gpsimd.dma_start(out=w1x[0][:], in_=w1[0, 512:DA, :])
            nc.scalar.dma_start(
                out=w1b[1][:],
                in_=w1[1, 0:512, :].rearrange("(a p) h -> p a h", p=128))
            nc.scalar.dma_start(out=w1x[1][:], in_=w1[1, 512:DA, :])
            for c in range(2):
                nc.gpsimd.dma_start(
                    out=w2b[c][0][:],
                    in_=w2[c, 0:512, :].rearrange("(a p) h -> p a h", p=128))
                nc.scalar.dma_start(
                    out=w2b[c][1][:],
                    in_=w2[c, 512:H, :].rearrange("(a p) h -> p a h", p=128))
            for c in range(2):
                eng = nc.gpsimd if c == 0 else nc.scalar
                eng.dma_start(
                    out=wht[c][:],
                    in_=whr[c].rearrange("(a p) h -> p a h", p=128))
                nc.gpsimd.dma_start(out=vt[c][:], in_=vecs[c])
                nc.gpsimd.dma_start(out=qtbv[c][:], in_=qtb[c])

            mt0 = wp.tile([128, 128], F32, tag="mt0", name="mt0")
            nc.vector.memset(mt0[:], 1.0 / H)
            mt = wp.tile([128, 128], F32R, tag="mt", name="mt")
            nc.vector.tensor_copy(mt[:], mt0[:])
            wmp = qps.tile([128, 128], F32, tag="qp", name="wmp")
            for _ in range(76):
                nc.tensor.matmul(wmp[:], mt[:], mt[:], start=True, stop=True)
            epst = wp.tile([128, 1], F32, tag="epst", name="epst")
            nc.vector.memset(epst[:], EPS)

            def b_ap(c, i, m):
                return vt[c][:, i, m:m + 1]

            def rsqrt(dst, src, bias):
                nc.scalar.activation(dst, src, AF.Abs_reciprocal_sqrt,
                                     bias=bias)

            def mm_block_gen(c, act, wts_of_m, nk, layer):
                zs = []
                sp = sps.tile([128, NT], F32, tag="sp", name="sp")
                pend = []

                def flush(upto):
                    while pend and pend[0][0] <= upto:
                        m, z2 = pend.pop(0)
                        nc.tensor.matmul(sp[:], mt[:], z2[:],
                                         start=(m == 0), stop=(m == NM - 1))

                for m in range(NM):
                    zpm = zps.tile([128, NT], F32, tag="zp", name="zp")
                    for k in range(nk):
                        nc.tensor.matmul(zpm[:], wts_of_m(k, m), act[k][:],
                                         start=(k == 0), stop=(k == nk - 1))
                    z2 = zp_.tile([128, NT], F32R, tag=f"z2_{m % 3}",
                                  name=f"z2_{m % 3}", bufs=1)
                    z = zp_.tile([128, NT], F32, tag=f"zs{m}", name=f"zs{m}")
                    if layer == 0:
                        nc.scalar.activation(z2[:], zpm[:], AF.Square)
                        nc.vector.tensor_copy(z[:], zpm[:])
                    else:
                        nc.scalar.activation(z[:], zpm[:], AF.Identity,
                                             bias=b_ap(c, 3, m))
                        nc.vector.tensor_mul(z2[:], z[:], z[:])
                    pend.append((m, z2))
                    flush(m - 2)
                    zs.append(z)
                flush(NM)
                return zs, sp

            def norm_block_gen(c, zs, sp, layer):
                g_i, be_i = (1, 2) if layer == 0 else (4, 5)
                rs = sp_.tile([128, NT], F32, tag="rs128", name="rs128")
                rsqrt(rs[:], sp[:], epst[:])
                hs = []
                for m in range(NM):
                    nc.vector.tensor_mul(zs[m][:], zs[m][:], rs[:])
                    ht = hp.tile([128, NT], F32R, tag=f"h{m}", name=f"h{m}")
                    nc.scalar.activation(ht[:], zs[m][:], AF.Relu,
                                         bias=b_ap(c, be_i, m),
                                         scale=b_ap(c, g_i, m))
                    hs.append(ht)
                return hs

            for bt in range(NBT):
                b0 = bt * NT
                xk = []
                off = 0
                for k in range(len(KL1)):
                    t = xp.tile([KL1[k], NT], F32R, tag=f"x{k}", name=f"x{k}")
                    nc.sync.dma_start(out=t[:],
                                      in_=xT[off:off + KL1[k], b0:b0 + NT])
                    xk.append(t)
                    off += KL1[k]
                st = {}
                for c in range(2):
                    st[c] = mm_block_gen(
                        c, xk, lambda k, m, c=c: w1_ap(c, k, m), len(KL1), 0)
                h1 = {}
                for c in range(2):
                    h1[c] = norm_block_gen(c, st[c][0], st[c][1], 0)
                for c in range(2):
                    st[c] = mm_block_gen(
                        c, h1[c], lambda k, m, c=c: w2_ap(c, k, m), NM, 1)
                for c in range(2):
                    ff = norm_block_gen(c, st[c][0], st[c][1], 1)
                    qp = qps.tile([64, NT], F32, tag="qp", name="qp")
                    for k in range(NM):
                        nc.tensor.matmul(qp[:], wht[c][:, k, :], ff[k][:],
                                         start=(k == 0), stop=(k == NM - 1))
                    qf = sp_.tile([64, NT], F32, tag="qf", name="qf", bufs=2)
                    nc.scalar.activation(qf[:], qp[:], AF.Identity,
                                         bias=qtbv[c][:])
                    nc.gpsimd.dma_start(out=out_q[c, :, b0:b0 + NT],
                                        in_=qf[:])
    nc.compile()
    return nc


def _build(unit_affine):
    nc = bacc.Bacc("TRN2", target_bir_lowering=False, debug=False,
                   num_devices=NCORES)
    if unit_affine:
        return _build_fast(nc)
    return _build_general(nc)


def _common_host(inputs):
    W_f1 = np.asarray(inputs["W_f1"], np.float32)
    b_f1 = np.asarray(inputs["b_f1"], np.float32)
    W_f2 = np.asarray(inputs["W_f2"], np.float32)
    b_f2 = np.asarray(inputs["b_f2"], np.float32)
    W_h = np.asarray(inputs["W_h"], np.float32)
    b_h = np.asarray(inputs["b_h"], np.float32)
    W_e1 = np.asarray(inputs["W_e1"], np.float32)
    b_e1 = np.asarray(inputs["b_e1"], np.float32)
    W_e2 = np.asarray(inputs["W_e2"], np.float32)
    b_e2 = np.asarray(inputs["b_e2"], np.float32)

    # transpose weights and fold the LN mean subtraction into them
    w1t = np.ascontiguousarray(W_f1.transpose(0, 2, 1))  # [2, D, H]
    w1c = w1t - w1t.mean(axis=2, keepdims=True)
    b1c = b_f1 - b_f1.mean(axis=1, keepdims=True)        # [2, H]
    w2t = np.ascontiguousarray(W_f2.transpose(0, 2, 1))  # [2, H, H]
    w2c = w2t - w2t.mean(axis=2, keepdims=True)
    b2c = b_f2 - b_f2.mean(axis=1, keepdims=True)        # [2, H]
    wh_feat = W_h[:, 0, :H]                              # [2, H]

    # tau embedding: batch-independent, tiny -> host
    tau = (np.linspace(0.0, 1.0, NQ + 1, dtype=np.float32)[:-1]
           + np.float32(1.0 / (2 * NQ)))[:, None]        # [NQ, 1]
    qtb = np.empty((2, 64, 1), np.float32)
    for c in range(2):
        te = np.maximum(tau @ W_e1[c].T + b_e1[c], 0.0) @ W_e2[c].T + b_e2[c]
        qtb[c, :, 0] = te @ W_h[c, 0, H:] + b_h[c, 0]
    return w1c, b1c, w2c, b2c, wh_feat, qtb


def _prep_fast(inputs):
    state = np.asarray(inputs["state"], np.float32)
    action = np.asarray(inputs["action"], np.float32)
    w1c, b1c, w2c, b2c, wh_feat, qtb = _common_host(inputs)
    bf = np.dtype("bfloat16") if hasattr(np, "bfloat16") else None
    import ml_dtypes
    bf = ml_dtypes.bfloat16

    # x, slab-major bf16, x8 scale: xs[p, s, b] = 8*x[s*128+p, b]
    xT = np.ascontiguousarray(state.T * 8.0)             # [512, B]
    xs = xT.reshape(4, 128, B).transpose(1, 0, 2).astype(bf)  # [128, 4, B]
    aT = np.ascontiguousarray(action.T * 8.0)            # [64, B]
    xa = np.concatenate([aT, aT], axis=0).astype(bf)     # [128, B] dup

    def slab(w):                                         # [512, H]->[128,4,H]
        return np.ascontiguousarray(
            w.reshape(4, 128, H).transpose(1, 0, 2))

    w1s = np.stack([slab(8.0 * w1c[c, :512]) for c in range(2)]).astype(bf)
    # [2, 128, H]: per-critic action weights duplicated across both 64-row
    # strips (even m-tiles read rows 0-63, odd read 64-127)
    w1a = np.stack([np.concatenate([8.0 * w1c[c, 512:]] * 2, axis=0)
                    for c in range(2)]).astype(bf)
    w2 = np.stack([np.stack([slab(8.0 * w2c[c, 512 * j:512 * (j + 1)])
                             for j in range(2)]) for c in range(2)]).astype(bf)
    whr = np.ascontiguousarray(np.broadcast_to(
        (8.0 * wh_feat).reshape(2, NM, 128, 1).transpose(0, 2, 1, 3),
        (2, 128, NM, 64)).copy()).astype(bf)

    def as_pm(v):                                        # [2,H] -> [2,128,NM]
        return v.reshape(2, NM, 128).transpose(0, 2, 1)

    vecs = np.ascontiguousarray(np.stack(
        [as_pm(4.0 * b1c), as_pm(64.0 * b1c),
         as_pm(4.0 * b2c), as_pm(64.0 * b2c)],
        axis=1).transpose(0, 2, 1, 3))                   # [2, 128, 4, NM]

    shared = {"w1s": w1s, "w1a": np.ascontiguousarray(w1a),
              "w2": w2, "wh": whr,
              "vecs": vecs, "qtb": qtb}
    return xs, xa, shared


def _prep_general(inputs):
    state = np.ascontiguousarray(inputs["state"], dtype=np.float32)
    action = np.ascontiguousarray(inputs["action"], dtype=np.float32)
    g1 = np.asarray(inputs["g1"], np.float32)
    beta1 = np.asarray(inputs["beta1"], np.float32)
    g2 = np.asarray(inputs["g2"], np.float32)
    beta2 = np.asarray(inputs["beta2"], np.float32)
    w1c, b1c, w2c, b2c, wh_feat, qtb = _common_host(inputs)

    x = np.concatenate(
        [state, action, np.ones((B, 1), np.float32)], axis=1)  # [B, 577]
    xT = np.ascontiguousarray(x.T)                             # [577, B]
    w1a = np.concatenate([w1c, b1c[:, None, :]], axis=1)       # [2, DA, H]

    def as_pm(v):
        return v.reshape(2, NM, 128).transpose(0, 2, 1)

    vecs = np.ascontiguousarray(np.stack(
        [as_pm(b2c), as_pm(g1), as_pm(beta1),
         as_pm(b2c), as_pm(g2), as_pm(beta2)],
        axis=1).transpose(0, 2, 1, 3))                   # [2, 128, 6, NM]
    whr = np.ascontiguousarray(
        np.broadcast_to(wh_feat[:, :, None], (2, H, 64)).copy())

    shared = {"w1": np.ascontiguousarray(w1a),
              "w2": np.ascontiguousarray(w2c),
              "whr": whr, "vecs": vecs, "qtb": qtb}
    return xT, shared


def kernel(**inputs):
    global _LAST_RESULT
    g1 = np.asarray(inputs["g1"], np.float32)
    beta1 = np.asarray(inputs["beta1"], np.float32)
    g2 = np.asarray(inputs["g2"], np.float32)
    beta2 = np.asarray(inputs["beta2"], np.float32)
    unit_affine = (np.all(g1 == 1.0) and np.all(beta1 == 0.0)
                   and np.all(g2 == 1.0) and np.all(beta2 == 0.0))

    key = ("nc", unit_affine)
    if key not in _CACHE:
        _CACHE[key] = _build(unit_affine)
    nc = _CACHE[key]

    in_maps = []
    if unit_affine:
        xs, xa, shared = _prep_fast(inputs)
        for c in range(NCORES):
            m = dict(shared)
            sh = xs[:, :, c * BSH:(c + 1) * BSH]         # [128, 4, BSH]
            m["xs"] = np.ascontiguousarray(              # [NBT, 128, 4, NT]
                sh.reshape(128, 4, NBT, NT).transpose(2, 0, 1, 3))
            sa = xa[:, c * BSH:(c + 1) * BSH]
            m["xa"] = np.ascontiguousarray(              # [NBT, 128, NT]
                sa.reshape(128, NBT, NT).transpose(1, 0, 2))
            in_maps.append(m)
    else:
        xT, shared = _prep_general(inputs)
        for c in range(NCORES):
            m = dict(shared)
            m["xT"] = np.ascontiguousarray(xT[:, c * BSH:(c + 1) * BSH])
            in_maps.append(m)

    trace = bool(os.environ.get("KERNEL_TRACE"))
    res = run_bass_kernel_spmd(nc, in_maps, list(range(NCORES)), trace=trace)
    _LAST_RESULT = res

    q = np.concatenate([res.results[i]["out_q"] for i in range(NCORES)],
                       axis=2)                           # [2, NQ, B]
    q = np.ascontiguousarray(q.transpose(0, 2, 1))       # [2, B, NQ]
    return q[0], q[1]
